# revision 11
# baseline (speedup 1.0000x reference)
"""Trainium2 Bass kernel for nn_Encoder_70781061038947.

Math: row b's output depends on x[b, :] only through its 16 sign bits
(root k has radius R if x[b,k] > 0 else 1/R, phase shuffle_vector[k]).
Evaluate the monic degree-16 polynomial at the 17th roots of unity in
LOG space: log P(t_m) = const_m + sum_k s_k c_mk with s_k = sign(x_bk),
c_mk = (Log(t_m - R e^{i th_k}) - Log(t_m - e^{i th_k}/R)) / 2.  That
is a single [16 x 34] contraction per row -> one bf16 hi/lo matmul pair
per 1024-row chunk (block-diagonal 8x copy of the C table).  Then
P = K_m * exp(lr) * (cos li, sin li) with the constant K_m folded into
the inverse-DFT matrix, the trig computed on the Scalar engine after a
mod-1 range reduction on Vector, and coefficients recovered with one
transposed-value matmul per 3-subtile group.  Normalization happens on
the coefficients directly (||c||, Parseval not needed).

Per 1024-row chunk: 9 PE instructions (1 sign transpose, 2 main matmul
hi/lo, 3 value transposes, 3 inverse-DFT matmuls), 5 ACT, 6 DVE,
3 GpSimd — versus 35 PE instructions for the one-hot/gather approach.

Sharding: pure data parallel over B across 8 cores (32768 rows each);
the small tables derived from shuffle_vector are replicated inputs.
"""

import numpy as np
import ml_dtypes

import concourse.bacc as bacc
import concourse.bass as bass
import concourse.mybir as mybir
import concourse.bass_utils as bass_utils
import concourse.tile as tile

B = 262144
K = 16
M = 17                      # evaluation points (17th roots of unity)
W = 2 * M                   # 34 f32 per output row
NCORES = 8
RPC = B // NCORES           # 32768 rows per core
P = 128
CPB = RPC // P              # 256 rows per partition
TPC = 8                     # subtiles (row-columns) per chunk
NCHUNK = CPB // TPC         # 32 chunks
FT = TPC * K                # 128: free width of one chunk of x
FO = TPC * W                # 272: free width of one chunk of out

_cached = None


def _tables(shuffle_vector: np.ndarray):
    sv = np.asarray(shuffle_vector, dtype=np.float64)
    R = np.sqrt(1.0 + np.sin(np.pi / K))
    t = np.exp(2j * np.pi * np.arange(M) / M)
    bf16 = ml_dtypes.bfloat16

    zhi = R * np.exp(1j * sv)
    zlo = np.exp(1j * sv) / R
    a = np.log(t[None, :] - zhi[:, None])          # (K, M)
    b = np.log(t[None, :] - zlo[:, None])
    c = (a - b) / 2
    Km = np.exp(((a + b) / 2).sum(axis=0))         # (M,)

    # C table: cols 0..16 -> Re c; cols 17..33 -> Im c / 2pi (mod-1 trick)
    Cmat = np.concatenate([c.real, c.imag / (2 * np.pi)], axis=1)  # (16, 34)

    # block-diagonal 8x copy: row 16u+k, col 34u+q
    Cbig = np.zeros((P, FO), np.float64)
    for u in range(TPC):
        Cbig[u * K:(u + 1) * K, u * W:(u + 1) * W] = Cmat
    chi = Cbig.astype(bf16)
    clo = (Cbig - chi.astype(np.float64)).astype(bf16)

    # inverse DFT with K_m folded: c_d = sum_m Q_m * (K_m w_md),
    # w_md = exp(-2pi i (K-d) m / 17) / 17; rows [Qre(17); Qim(17)],
    # cols re/im interleaved.
    w = np.exp(-2j * np.pi * ((K - np.arange(M)[None, :])
                              * np.arange(M)[:, None]) / M) / M
    WKc = Km[:, None] * w
    W2K = np.zeros((W, W), np.float64)
    W2K[:M, 0::2] = WKc.real
    W2K[:M, 1::2] = WKc.imag
    W2K[M:, 0::2] = -WKc.imag
    W2K[M:, 1::2] = WKc.real

    w2k3 = np.zeros((3 * W, 3 * W), np.float64)
    for j in range(3):
        w2k3[j * W:(j + 1) * W, j * W:(j + 1) * W] = W2K
    w2k2 = np.zeros((2 * W, 2 * W), np.float64)
    for j in range(2):
        w2k2[j * W:(j + 1) * W, j * W:(j + 1) * W] = W2K

    ident_bf = np.eye(P, dtype=bf16)
    pihalf = np.full((P, 1), np.pi / 2, np.float32)

    return {
        "chi": chi,
        "clo": clo,
        "w2k3": w2k3.astype(bf16),
        "w2k2": w2k2.astype(bf16),
        "identb": ident_bf,
        "pihalf": pihalf,
    }


def _build_module(rpc=RPC):
    cpb = rpc // P
    nchunk = cpb // TPC
    f32 = mybir.dt.float32
    bf = mybir.dt.bfloat16
    AF = mybir.ActivationFunctionType
    OP = mybir.AluOpType
    PI = float(np.pi)

    nc = bacc.Bacc("TRN2", target_bir_lowering=False, debug=False)
    x_d = nc.dram_tensor("x", [rpc, K], bf, kind="ExternalInput")
    chi_d = nc.dram_tensor("chi", [P, FO], bf, kind="ExternalInput")
    clo_d = nc.dram_tensor("clo", [P, FO], bf, kind="ExternalInput")
    w2k3_d = nc.dram_tensor("w2k3", [3 * W, 3 * W], bf, kind="ExternalInput")
    w2k2_d = nc.dram_tensor("w2k2", [2 * W, 2 * W], bf, kind="ExternalInput")
    identb_d = nc.dram_tensor("identb", [P, P], bf, kind="ExternalInput")
    pihalf_d = nc.dram_tensor("pihalf", [P, 1], f32, kind="ExternalInput")
    out_d = nc.dram_tensor("out", [rpc, W], f32, kind="ExternalOutput")

    # row (p*cpb + c) -> partition p, column c
    x_v = x_d.ap().rearrange("(p c) k -> p (c k)", p=P)      # [128, cpb*16]
    out_v = out_d.ap().rearrange("(p c) e -> p (c e)", p=P)  # [128, cpb*34]

    with tile.TileContext(nc) as tc:
        with (
            tc.tile_pool(name="const", bufs=1) as cp,
            tc.tile_pool(name="sb", bufs=4) as sp,
            tc.tile_pool(name="ps", bufs=1, space="PSUM") as pp,
        ):
            chi_sb = cp.tile([P, FO], bf)
            nc.sync.dma_start(out=chi_sb[:], in_=chi_d.ap())
            clo_sb = cp.tile([P, FO], bf)
            nc.sync.dma_start(out=clo_sb[:], in_=clo_d.ap())
            w2k3_sb = cp.tile([3 * W, 3 * W], bf)
            nc.sync.dma_start(out=w2k3_sb[:], in_=w2k3_d.ap())
            w2k2_sb = cp.tile([2 * W, 2 * W], bf)
            nc.sync.dma_start(out=w2k2_sb[:], in_=w2k2_d.ap())
            identb = cp.tile([P, P], bf)
            nc.sync.dma_start(out=identb[:], in_=identb_d.ap())
            pihalf = cp.tile([P, 1], f32)
            nc.sync.dma_start(out=pihalf[:], in_=pihalf_d.ap())

            for ci in range(nchunk):
                x_sb = sp.tile([P, FT], bf, tag="x")
                nc.sync.dma_start(out=x_sb[:], in_=x_v[:, ci * FT:(ci + 1) * FT])

                # one [128,128] transpose: partition (16u+k), col p holds
                # x[row(p,u), k]; sign it into SBUF (+-1 bf16)
                xT = pp.tile([P, P], bf, tag="xT", bufs=2)
                nc.tensor.transpose(out=xT[:], in_=x_sb[:], identity=identb[:])
                sT = sp.tile([P, P], bf, tag="sT")
                nc.scalar.activation(out=sT[:], in_=xT[:], func=AF.Sign)

                # log-evals: L[p, (u, q)] = sum_k s_k C[k, q] (hi+lo bf16)
                L = pp.tile([P, FO], f32, tag="L", bufs=2)
                nc.tensor.matmul(out=L[:], lhsT=sT[:], rhs=chi_sb[:],
                                 start=True, stop=False)
                nc.tensor.matmul(out=L[:], lhsT=sT[:], rhs=clo_sb[:],
                                 start=False, stop=True)
                Lv = L[:].rearrange("p (u q) -> p u q", q=W)
                lr = Lv[:, :, 0:M]            # log|Q|
                ph = Lv[:, :, M:W]            # arg(Q) / 2pi

                er = sp.tile([P, TPC * M], f32, tag="er")
                er_v = er[:].rearrange("p (u m) -> p u m", m=M)
                nc.scalar.activation(out=er_v, in_=lr, func=AF.Exp)

                # range reduction: round() via the 1.5*2^23 magic constant
                # w = y - round(y) in [-.5,.5]; w2 = y - round(y+1/4) in [-.75,.25]
                MAGIC = float(1.5 * 2 ** 23)
                r1 = sp.tile([P, TPC * M], f32, tag="r1")
                nc.vector.tensor_scalar(
                    out=r1[:].rearrange("p (u m) -> p u m", m=M), in0=ph,
                    scalar1=MAGIC, scalar2=MAGIC, op0=OP.add, op1=OP.subtract)
                wred = sp.tile([P, TPC * M], f32, tag="wred")
                nc.vector.tensor_tensor(
                    out=wred[:].rearrange("p (u m) -> p u m", m=M), in0=ph,
                    in1=r1[:].rearrange("p (u m) -> p u m", m=M), op=OP.subtract)
                r2 = sp.tile([P, TPC * M], f32, tag="r2")
                nc.vector.tensor_scalar(
                    out=r2[:].rearrange("p (u m) -> p u m", m=M), in0=ph,
                    scalar1=MAGIC + 0.25, scalar2=MAGIC, op0=OP.add, op1=OP.subtract)
                vred = sp.tile([P, TPC * M], f32, tag="vred")
                nc.vector.tensor_tensor(
                    out=vred[:].rearrange("p (u m) -> p u m", m=M), in0=ph,
                    in1=r2[:].rearrange("p (u m) -> p u m", m=M), op=OP.subtract)

                # sin(li) = sin(2pi*w); cos(li) = sin(2pi*w2 + pi/2)
                sinli = sp.tile([P, TPC * M], f32, tag="sinli")
                nc.scalar.activation(out=sinli[:], in_=wred[:], func=AF.Sin,
                                     bias=0.0, scale=2.0 * PI)
                cosli = sp.tile([P, TPC * M], f32, tag="cosli")
                nc.scalar.activation(out=cosli[:], in_=vred[:], func=AF.Sin,
                                     bias=pihalf[:], scale=2.0 * PI)

                # Q values, bf16, packed [Qre(17) | Qim(17)] per subtile
                vc = sp.tile([P, FO], bf, tag="vc")
                vcv = vc[:].rearrange("p (u q) -> p u q", q=W)
                sin_v = sinli[:].rearrange("p (u m) -> p u m", m=M)
                cos_v = cosli[:].rearrange("p (u m) -> p u m", m=M)
                nc.gpsimd.tensor_tensor(out=vcv[:, :, 0:M], in0=er_v,
                                        in1=cos_v, op=OP.mult)
                nc.gpsimd.tensor_tensor(out=vcv[:, :, M:W], in0=er_v,
                                        in1=sin_v, op=OP.mult)

                # transpose values in subtile-groups of (3,3,2)
                vcT = pp.tile([3 * W, 3 * P], bf, tag="vcT", bufs=2)
                widths = [3 * W, 3 * W, 2 * W]
                for j, wdt in enumerate(widths):
                    nc.tensor.transpose(
                        out=vcT[0:wdt, j * P:(j + 1) * P],
                        in_=vc[:, j * 3 * W: j * 3 * W + wdt],
                        identity=identb[:])
                vcT_sb = sp.tile([3 * W, 3 * P], bf, tag="vcTs")
                nc.vector.tensor_copy(out=vcT_sb[:], in_=vcT[:])

                # block-diagonal inverse-DFT (K_m folded): row-major coeffs
                o_ps = pp.tile([P, FO], f32, tag="o", bufs=2)
                nc.tensor.matmul(
                    out=o_ps[:, 0:3 * W], lhsT=vcT_sb[0:3 * W, 0:P],
                    rhs=w2k3_sb[:], start=True, stop=True)
                nc.tensor.matmul(
                    out=o_ps[:, 3 * W:6 * W], lhsT=vcT_sb[0:3 * W, P:2 * P],
                    rhs=w2k3_sb[:], start=True, stop=True)
                nc.tensor.matmul(
                    out=o_ps[:, 6 * W:8 * W], lhsT=vcT_sb[0:2 * W, 2 * P:3 * P],
                    rhs=w2k2_sb[:], start=True, stop=True)

                # normalize from the coefficients: fac = sqrt(17 / sum c^2)
                sq = sp.tile([P, FO], f32, tag="sq")
                nc.scalar.activation(out=sq[:], in_=o_ps[:], func=AF.Square)
                S = sp.tile([P, TPC], f32, tag="S")
                nc.vector.tensor_reduce(
                    out=S[:], in_=sq[:].rearrange("p (u e) -> p u e", e=W),
                    axis=mybir.AxisListType.X, op=OP.add)
                rS = sp.tile([P, TPC], f32, tag="rS")
                nc.vector.reciprocal(out=rS[:], in_=S[:])
                fac = sp.tile([P, TPC], f32, tag="fac")
                nc.scalar.activation(out=fac[:], in_=rS[:], func=AF.Sqrt,
                                     bias=0.0, scale=float(M))

                out_sb = sp.tile([P, FO], f32, tag="osb")
                nc.vector.tensor_tensor(
                    out=out_sb[:].rearrange("p (u e) -> p u e", e=W),
                    in0=o_ps[:].rearrange("p (u e) -> p u e", e=W),
                    in1=fac[:].unsqueeze(2).to_broadcast([P, TPC, W]),
                    op=OP.mult)
                nc.scalar.dma_start(
                    out=out_v[:, ci * FO:(ci + 1) * FO], in_=out_sb[:])

    nc.compile()
    return nc


def kernel(x: np.ndarray, shuffle_vector: np.ndarray) -> np.ndarray:
    global _cached
    x = np.asarray(x)
    assert x.shape == (B, K), x.shape
    x_bf = np.ascontiguousarray(x.astype(ml_dtypes.bfloat16))

    tabs = _tables(shuffle_vector)
    if _cached is None:
        _cached = _build_module()
    nc = _cached

    shards = x_bf.reshape(NCORES, RPC, K)
    in_maps = [
        {"x": np.ascontiguousarray(shards[i]), **tabs}
        for i in range(NCORES)
    ]
    res = bass_utils.run_bass_kernel_spmd(nc, in_maps, core_ids=list(range(NCORES)))
    out = np.concatenate([res.results[i]["out"] for i in range(NCORES)], axis=0)
    return np.ascontiguousarray(out).view(np.complex64).reshape(B, M).astype(np.complex128)
